# revision 45
# baseline (speedup 1.0000x reference)
"""Causal self-attention with RoPE, tensor-parallel over heads on 8 TRN2 NeuronCores.

Model (from the reference):
    q/k/v = x @ W{q,k,v}.T          x: (1, 2048, 2048), 16 heads x 128 head_dim
    rope(q), rope(k)                half-rotation, 32 nonzero freqs
    causal softmax(q k^T / sqrt(128)) @ v
    out = (y / 3) @ Wo.T

Sharding: 2 heads per core. Each core computes its heads' q/k/v projections,
attention, and a partial c_proj (its 256 columns of the hd contraction);
the host sums the 8 partial outputs (the "all-reduce after c_proj").

Numerics (validated against the reference on the real inputs, final
max-err/absmax ~9.4e-3 vs the 2e-2 gate):
  - q/k projections: raw fp8e4m3 (x_hi, w*2^8) via DoubleRow matmuls
    (2x128 contraction per instruction at 0.5 cycles/row -> 4x PE rate).
    Score errors are damped because |scores| is small, so attention
    probabilities see only a small absolute perturbation.
  - v projection and c_proj: 3-term compensated fp8 (x_hi@w_hi + x_hi@w_lo
    + x_lo@w_hi), each term a DoubleRow matmul -> 1.33x PE rate with
    ~1e-3 final error. These feed the output linearly, so raw fp8 would
    blow the error budget.
  - scores: DoubleRow fp8 over dh-half pairs at 0.5 cycles/row (2x): q/k
    are folded into a [64, 2] half-pair layout by SBUF->SBUF DMAs, padded
    to 128 rows (ktf upper lanes zeroed once; qc upper lanes junk) because
    the device path rejects 64-row DoubleRow tiles. Causal diagonal
    trimmed at 128 granularity.
  - P (exp scores) and V: bf16 -> PV matmuls at full rate any width, DVE
    masking/accumulation in 2x mode. P in fp8 would blow the budget (the
    quantization hits y at full weight), so PV stays non-DoubleRow.
  - RoPE roll (partition rotation by 64) via two SBUF->SBUF DMAs instead
    of a PE matmul; rope muls on DVE in bf16 2x with broadcast tables.
  - Output partials in bf16 (summed across cores on the host in f64).

Schedule: the attention j-loop is ACT-bound (exp), so each span drains a
filler-thunk queue (next chunk's q/k/v projections, previous chunk's
c_proj tiles) one thunk per j-tile to keep the PE fed; PV trails scores
by one stage; the m==0 diagonal PV closes the ots psum group last and
full-width; the denominator/reciprocal/y-split tail runs per 128-column
piece so c_proj starts immediately; all DMA counts are minimized (one
per weight, 8 x-pieces and one output row-DMA per token tile) because
the shared HWDGE costs ~625ns per descriptor-gen.

Layout: everything transposed so the contraction dim is on partitions;
scores computed as S^T so P^T @ V needs no transposes; softmax without
max-subtraction (scores are provably tiny); denominator via bf16 vecsum
accumulation (DVE+GPSIMD) + an all-(1/16) matmul, the 16 folded back in
the reciprocal, giving y*16 which fp8 splits cleanly for c_proj.
"""

import numpy as np

T = 2048
D = 2048
H = 16
DH = 128
N_CORES = 8
H_LOC = H // N_CORES          # heads per core = 2
HD_LOC = H_LOC * DH           # local head dims = 256
TCH = 512                     # query-chunk width
N_CH = T // TCH               # 4 chunks
KO = D // 128                 # 16 contraction subtiles
XP = 8                        # xT streamed in pieces of 8 k-subtiles
KP = KO // 2                  # 8 DoubleRow pairs over the contraction
SCALE = (DH ** 0.5) / DH      # 1/sqrt(128)
WSC = 256.0                   # fp8 weight pre-scale (2^8)
YSC = 16.0                    # y pre-scale folded into the reciprocal

_CACHE = {}


def build_program():
    """Build (once) the single-core Bass program shared by all 8 cores."""
    if "nc" in _CACHE:
        return _CACHE["nc"]

    from contextlib import ExitStack

    import concourse.bacc as bacc
    import concourse.mybir as mybir
    import concourse.tile as tile
    from concourse.bass import broadcast_tensor_aps

    f32 = mybir.dt.float32
    bf16 = mybir.dt.bfloat16
    f8 = mybir.dt.float8e4
    EXP = mybir.ActivationFunctionType.Exp
    COPY = mybir.ActivationFunctionType.Copy
    DR = mybir.MatmulPerfMode.DoubleRow

    nc = bacc.Bacc("TRN2", target_bir_lowering=False)

    # weights arrive pre-rearranged to partition-major layouts so each loads
    # with ONE full-rate DMA (4KB+ contiguous per partition)
    x8h_d = nc.dram_tensor("x8h", (D, T), f8, kind="ExternalInput")
    x8l_d = nc.dram_tensor("x8l", (D, T), f8, kind="ExternalInput")
    wq_d = nc.dram_tensor("wq8", (128, KO * HD_LOC), f8, kind="ExternalInput")
    wk_d = nc.dram_tensor("wk8", (128, KO * HD_LOC), f8, kind="ExternalInput")
    wvh_d = nc.dram_tensor("wv8h", (128, KO * HD_LOC), f8, kind="ExternalInput")
    wvl_d = nc.dram_tensor("wv8l", (128, KO * HD_LOC), f8, kind="ExternalInput")
    woh_d = nc.dram_tensor("wo8h", (128, H_LOC * D), f8, kind="ExternalInput")
    wol_d = nc.dram_tensor("wo8l", (128, H_LOC * D), f8, kind="ExternalInput")
    ct_d = nc.dram_tensor("ctab", (128, T), bf16, kind="ExternalInput")
    st_d = nc.dram_tensor("stab", (128, T), bf16, kind="ExternalInput")
    ones_d = nc.dram_tensor("ones", (128, 128), bf16, kind="ExternalInput")
    tri_d = nc.dram_tensor("tri", (128, 128), bf16, kind="ExternalInput")
    out_d = nc.dram_tensor("outp", (T, D), bf16, kind="ExternalOutput")

    x8h_r = x8h_d[:].rearrange("(ko p) t -> p ko t", p=128)
    x8l_r = x8l_d[:].rearrange("(ko p) t -> p ko t", p=128)
    wq_r = wq_d[:].rearrange("p (ko m) -> p ko m", ko=KO)
    wk_r = wk_d[:].rearrange("p (ko m) -> p ko m", ko=KO)
    wvh_r = wvh_d[:].rearrange("p (ko m) -> p ko m", ko=KO)
    wvl_r = wvl_d[:].rearrange("p (ko m) -> p ko m", ko=KO)
    woh_r = woh_d[:].rearrange("p (h d) -> p h d", h=H_LOC)
    wol_r = wol_d[:].rearrange("p (h d) -> p h d", h=H_LOC)

    with tile.TileContext(nc) as tc, ExitStack() as ctx:
        persist = ctx.enter_context(tc.tile_pool(name="persist", bufs=1))
        qpool = ctx.enter_context(tc.tile_pool(name="qpool", bufs=2))
        ypool = ctx.enter_context(tc.tile_pool(name="ypool", bufs=2))
        xpool = ctx.enter_context(tc.tile_pool(name="xpool", bufs=6))
        ptpool = ctx.enter_context(tc.tile_pool(name="ptpool", bufs=4))
        rtmp = ctx.enter_context(tc.tile_pool(name="rtmp", bufs=2))
        spool = ctx.enter_context(tc.tile_pool(name="spool", bufs=2))
        opool = ctx.enter_context(tc.tile_pool(name="opool", bufs=6))
        psum_p = ctx.enter_context(tc.tile_pool(name="psum_p", bufs=2, space="PSUM"))
        psum_mix = ctx.enter_context(tc.tile_pool(name="psum_mix", bufs=2, space="PSUM"))
        psum_ot = ctx.enter_context(tc.tile_pool(name="psum_ot", bufs=2, space="PSUM"))

        def ps_tile(pool=None):
            return (pool or psum_p).tile([128, TCH], f32, tag="ps", name="ps")

        def mix_tile():
            return psum_mix.tile([128, H_LOC, TCH], f32, tag="mix", name="mix")

        # --- resident tensors ---
        w_q = persist.tile([128, KO, HD_LOC], f8, tag="w_q")
        w_k = persist.tile([128, KO, HD_LOC], f8, tag="w_k")
        w_vh = persist.tile([128, KO, HD_LOC], f8, tag="w_vh")
        w_vl = persist.tile([128, KO, HD_LOC], f8, tag="w_vl")
        w_oh = persist.tile([128, H_LOC, D], f8, tag="w_oh")
        w_ol = persist.tile([128, H_LOC, D], f8, tag="w_ol")
        ktf = persist.tile([128, H_LOC, 2, T], f8, tag="ktf")
        vt = persist.tile([128, KO, HD_LOC], bf16, tag="vt")
        ctab = persist.tile([128, 1, T], bf16, tag="ctab")
        stab = persist.tile([128, 1, T], bf16, tag="stab")
        ones = persist.tile([128, 128], bf16, tag="ones")
        tri = persist.tile([128, 128], bf16, tag="tri")

        def issue_part(c, part):
            """Queue the hi or lo x-piece DMAs for chunk c."""
            cs = c * TCH
            src_r = x8h_r if part == "hi" else x8l_r
            tiles = []
            for kp in range(KO // XP):
                ksl = slice(kp * XP, (kp + 1) * XP)
                xt = xpool.tile([128, XP, TCH], f8, tag="x" + part[0],
                                name="x" + part[0])
                nc.sync.dma_start(xt[:], src_r[:, ksl, cs:cs + TCH])
                tiles.append(xt)
            return tiles

        def issue_x(c):
            return (issue_part(c, "hi"), issue_part(c, "lo"))

        def piece_pair(pieces, xi, kp, tsl=slice(None)):
            """[128, 2, *] DoubleRow operand view for ko-pair kp from XP-wide tiles."""
            o = (kp * 2) % XP
            return pieces[xi][kp * 2 // XP][:, o:o + 2, tsl]

        def proj_rope(c, pre, dst, cs, cp_act):
            """RoPE for src `pre` -> folded fp8 dst (qc slot layout / ktf)."""
            rolled = rtmp.tile([128, H_LOC, TCH], bf16, tag="rolled",
                               name="rolled")
            nc.sync.dma_start(rolled[0:64, :, :], pre[64:128, :, :])
            nc.sync.dma_start(rolled[64:128, :, :], pre[0:64, :, :])
            a = rtmp.tile([128, H_LOC, TCH], bf16, tag="ra", name="ra")
            b = rtmp.tile([128, H_LOC, TCH], bf16, tag="rb", name="rb")
            _, cb = broadcast_tensor_aps(a[:, :, :], ctab[:, 0:1, cs:cs + TCH])
            _, sb = broadcast_tensor_aps(a[:, :, :], stab[:, 0:1, cs:cs + TCH])
            nc.vector.tensor_mul(out=a[:], in0=pre[:], in1=cb)
            nc.vector.tensor_mul(out=b[:], in0=rolled[:], in1=sb)
            # fold 128 dh-lanes into [64, 2] half-pairs for the padded
            # 128-row DoubleRow scores: ktf upper lanes stay zero, qc upper
            # lanes get junk (0 * junk contributes 0). The q add writes slot
            # 0 of all heads directly; shift halves move by SBUF->SBUF DMA.
            if dst is not ktf:
                nc.vector.tensor_add(out=dst[:, :, 0, :], in0=a[:], in1=b[:])
                nc.sync.dma_start(dst[0:64, :, 1, :], dst[64:128, :, 0, :])
                nc.sync.dma_start(dst[64:128, :, 1, :], dst[64:128, :, 0, :])
            else:
                tmp8 = rtmp.tile([128, H_LOC, TCH], f8, tag="t8", name="t8")
                nc.vector.tensor_add(out=tmp8[:], in0=a[:], in1=b[:])
                nc.sync.dma_start(ktf[0:64, :, 0, cs:cs + TCH],
                                  tmp8[0:64, :, :])
                nc.sync.dma_start(ktf[0:64, :, 1, cs:cs + TCH],
                                  tmp8[64:128, :, :])

        def proj_group(w_sb, pre, h, pieces, cp_act):
            ps = ps_tile()
            for kp in range(KP):
                nc.tensor.matmul(
                    ps,
                    lhsT=w_sb[:, kp * 2:(kp + 1) * 2, h * 128:(h + 1) * 128],
                    rhs=piece_pair(pieces, 0, kp),
                    start=(kp == 0),
                    stop=(kp == KP - 1),
                    perf_mode=DR,
                )
            if cp_act or w_sb is w_q:
                nc.scalar.activation(out=pre[:, h, :], in_=ps, func=COPY,
                                     scale=1.0 / WSC)
            else:
                nc.vector.tensor_scalar_mul(pre[:, h, :], ps, 1.0 / WSC)

        def v_group(c, tp, pieces):
            """3-term compensated fp8 v projection for token-tile pair tp."""
            vps = ps_tile()
            for tt in (2 * tp, 2 * tp + 1):
                gt = c * (TCH // 128) + tt
                tsl = slice(tt * 128, (tt + 1) * 128)
                ps = vps[:, (tt % 2) * HD_LOC:(tt % 2 + 1) * HD_LOC]
                n = 3 * KP
                i = 0
                for xi, wv in ((0, w_vh), (0, w_vl), (1, w_vh)):
                    for kp in range(KP):
                        nc.tensor.matmul(
                            ps,
                            lhsT=piece_pair(pieces, xi, kp, tsl),
                            rhs=wv[:, kp * 2:(kp + 1) * 2, :],
                            start=(i == 0),
                            stop=(i == n - 1),
                            perf_mode=DR,
                        )
                        i += 1
                nc.vector.tensor_scalar_mul(vt[:, gt, :], ps, 1.0 / WSC)

        def proj_q_thunks(c, pieces, cp_act=False):
            """Thunks: q projection (2 psum groups) + rope. Returns (qc,
            thunks)."""
            cs = c * TCH
            qc = qpool.tile([128, H_LOC, 2, TCH], f8, tag="qc", name="qc")
            pre = rtmp.tile([128, H_LOC, TCH], bf16, tag="pre", name="pre")
            thunks = [
                lambda h=h: proj_group(w_q, pre, h, pieces, cp_act)
                for h in range(H_LOC)
            ]
            thunks.append(lambda: proj_rope(c, pre, qc, cs, cp_act))
            return qc, thunks

        def proj_kv_thunks(c, pieces, cp_act=False):
            """Thunks: k projection + v projection + k rope (rope last so
            its DVE chain doesn't gate the v psum ring)."""
            cs = c * TCH
            pre = rtmp.tile([128, H_LOC, TCH], bf16, tag="pre", name="pre")
            thunks = [
                lambda h=h: proj_group(w_k, pre, h, pieces, cp_act)
                for h in range(H_LOC)
            ]
            thunks += [lambda tp=tp: v_group(c, tp, pieces) for tp in range(2)]
            thunks.append(lambda: proj_rope(c, pre, ktf, cs, cp_act))
            return thunks

        def attn_span(q0, W, qc, off, yc, fillers=()):
            """Causal attention for queries [q0, q0+W), heads interleaved.

            q0 must be 128-aligned; W in {256, 512}. qc holds the chunk's
            roped queries (fp8); off is q0's offset within qc/yc. The
            attention j-loop is ACT-bound (exp), so `fillers` (thunks
            emitting independent PE-heavy work: next chunk's projections,
            previous chunk's cproj tiles) are drained one per j-tile to keep
            the PE fed; leftovers drain before the denominator."""
            d0 = q0 // 128          # first diagonal j-tile
            n_jt = d0 + W // 128
            ots = [ps_tile(psum_ot) for _ in range(H_LOC)]
            vecsums = [spool.tile([128, H_LOC, TCH], bf16,
                                  tag=f"vecsum{par}", name="vecsum")
                       for par in range(2)]
            prev, held, started = None, None, [False]
            fillers = list(fillers)
            jt_lo, jt_hi = 0, n_jt

            def pv(p, stop=False):
                jt, pt, lo = p
                start = not started[0]
                started[0] = True
                # (lo, width, start, stop, skip_group_check) sub-issues.
                # start/stop must run the sim's group bookkeeping; interior
                # partial-width accumulates skip it. For the d0==0 span the
                # start was partial-width, leaving bytes [0:128) pending-zero:
                # the closing full-width PV must split into the two
                # uniformly-pending regions to satisfy the interp.
                if stop and d0 == 0:
                    parts = ((128, W - 128, False, False, True),
                             (0, 128, False, True, False))
                else:
                    parts = ((lo, W - lo, start, stop,
                              lo > 0 and not start and not stop),)
                for plo, pw, st, sp, skip in parts:
                    for h in range(H_LOC):
                        nc.tensor.matmul(
                            ots[h][:, plo:plo + pw],
                            lhsT=vt[:, jt, h * 128:(h + 1) * 128],
                            rhs=pt[:, h, plo:plo + pw],
                            start=st,
                            stop=sp,
                            skip_group_check=skip,
                        )

            for jt in range(jt_lo, jt_hi):
                pair = mix_tile()
                m = jt - d0
                # diagonal block: cols < 128m fully masked -- never written,
                # never read (partial-width ops; fp8/bf16 have no narrow-
                # matmul penalty, so trim at full 128 granularity)
                lo = 128 * m if m > 0 else 0
                for h in range(H_LOC):
                    nc.tensor.matmul(
                        pair[:, h, lo:W],
                        lhsT=ktf[:, h, :, jt * 128:(jt + 1) * 128],
                        rhs=qc[:, h, :, off + lo:off + W],
                        start=True,
                        stop=True,
                        perf_mode=DR,
                    )
                # the m==0 diagonal tile's pt is held out of the ring: its PV
                # (naturally full-width) is issued LAST with stop=True, so the
                # psum group closes with a write that overlaps every later
                # per-piece read of ots (correct ordering + closed group)
                pt = ptpool.tile([128, H_LOC, TCH], bf16,
                                 tag="ptd" if m == 0 else "pt", name="pt")
                # both heads in ONE activation call (strided AP when lo > 0)
                nc.scalar.activation(out=pt[:, :, lo:W], in_=pair[:, :, lo:W],
                                     func=EXP, scale=SCALE)
                for h in range(H_LOC):
                    if m >= 0:
                        nc.vector.tensor_mul(
                            out=pt[:, h, 128 * m:128 * (m + 1)],
                            in0=pt[:, h, 128 * m:128 * (m + 1)],
                            in1=tri[:],
                        )
                # spans starting at q0=0: jt==1 is diagonal (cols < 128
                # unwritten), so a full-width init copy would ingest
                # garbage -- single DVE accumulator there. Other spans give
                # every 4th j-tile to GPSIMD (its adds run at ~1/4 DVE's bf16
                # rate, so an even split would stall the pt ring), and the
                # last two j-tiles stay on DVE so the denominator matmuls
                # never wait on a trailing GPSIMD add.
                par = (1 if jt % 4 == 1 else 0) if (d0 >= 2 and jt < n_jt - 2) else 0
                vs = vecsums[par]
                eng = nc.vector if par == 0 else nc.gpsimd
                if jt < (2 if d0 >= 2 else 1):
                    eng.tensor_copy(out=vs[:, :, :W], in_=pt[:, :, :W])
                else:
                    eng.tensor_add(out=vs[:, :, lo:W], in0=vs[:, :, lo:W],
                                   in1=pt[:, :, lo:W])
                # software pipeline: PV for the PREVIOUS j-tile, so the PE
                # never waits on the exp/mask it just issued
                if m == 0:
                    held = (jt, pt, lo)
                else:
                    if prev is not None:
                        pv(prev)
                    prev = (jt, pt, lo)
                if fillers:
                    fillers.pop(0)()
            pv(prev)
            pv(held, stop=True)
            while fillers:
                fillers.pop(0)()
            # denominator: all-(1/16) matmul -> column sums/16 on all
            # partitions; the 16 resurfaces via the reciprocal so yc = 16*y.
            # The whole tail (den -> recip -> y -> fp8 split) runs per
            # 128-column piece so cproj's token-tile tt can start as soon as
            # piece tt is done instead of after the full-width chain.
            den = mix_tile()
            recipb = rtmp.tile([128, H_LOC, TCH], f32, tag="recipb",
                               name="recipb")
            y8h = ypool.tile([128, H_LOC, TCH], f8, tag="y8h", name="y8h")
            y8l = ypool.tile([128, H_LOC, TCH], f8, tag="y8l", name="y8l")
            for p in range(W // 128):
                pp = slice(p * 128, (p + 1) * 128)
                for h in range(H_LOC):
                    if d0 >= 2:
                        nc.tensor.matmul(den[:, h, pp], lhsT=ones,
                                         rhs=vecsums[0][:, h, pp],
                                         start=True, stop=False)
                        nc.tensor.matmul(den[:, h, pp], lhsT=ones,
                                         rhs=vecsums[1][:, h, pp],
                                         start=False, stop=True)
                    else:
                        nc.tensor.matmul(den[:, h, pp], lhsT=ones,
                                         rhs=vecsums[0][:, h, pp],
                                         start=True, stop=True)
                nc.vector.reciprocal(out=recipb[:, :, pp], in_=den[:, :, pp])
                for h in range(H_LOC):
                    nc.vector.tensor_mul(out=yc[:, h, pp], in0=ots[h][:, pp],
                                         in1=recipb[:, h, pp])
                nc.gpsimd.tensor_copy(out=y8h[:, :, pp], in_=yc[:, :, pp])
                nc.gpsimd.tensor_sub(out=y8l[:, :, pp], in0=yc[:, :, pp],
                                     in1=y8h[:, :, pp])
            return (y8h, y8l)

        def cproj_tile(q0, tt, y8, split_copies):
            """Partial c_proj for token-tile tt: 4x 512-col psum_p groups
            (never touching the attention's mix ring), gathered into one ob
            tile and one full-row output DMA."""
            y8h, y8l = y8
            gt = q0 // 128 + tt
            tsl = slice(tt * 128, (tt + 1) * 128)
            ob = opool.tile([128, 4, 512], bf16, tag="ob", name="ob")
            for nck in range(4):
                ps = ps_tile()[:, 0:512]
                dsl = slice(nck * 512, (nck + 1) * 512)
                for yy, ww, st, sp in ((y8h, w_oh, True, False),
                                       (y8h, w_ol, False, False),
                                       (y8l, w_oh, False, True)):
                    nc.tensor.matmul(
                        ps,
                        lhsT=yy[:, :, tsl],
                        rhs=ww[:, :, dsl],
                        start=st,
                        stop=sp,
                        perf_mode=DR,
                    )
                if nck % 4 == 3 or (split_copies and nck % 2 == 1):
                    nc.vector.tensor_scalar_mul(ob[:, nck, :], ps,
                                                1.0 / (WSC * YSC))
                else:
                    nc.scalar.activation(out=ob[:, nck, :], in_=ps, func=COPY,
                                         scale=1.0 / (WSC * YSC))
                if split_copies and nck % 2 == 1:
                    # final chunk: flush each half-row as soon as it's ready
                    # so the run doesn't end on a long serial DMA drain
                    nc.sync.dma_start(
                        out_d[gt * 128:(gt + 1) * 128,
                              (nck - 1) * 512:(nck + 1) * 512],
                        ob[:, nck - 1:nck + 1, :].rearrange("p a b -> p (a b)"),
                    )
            if not split_copies:
                nc.sync.dma_start(
                    out_d[gt * 128:(gt + 1) * 128, :],
                    ob[:].rearrange("p a b -> p (a b)"),
                )

        def cproj_thunks(q0, y8, split_copies=False, load_w=False):
            thunks = []
            if load_w:
                def _w():
                    nc.sync.dma_start(w_oh[:], woh_r)
                    nc.sync.dma_start(w_ol[:], wol_r)
                thunks.append(_w)
            thunks += [
                lambda tt=tt: cproj_tile(q0, tt, y8, split_copies)
                for tt in range(TCH // 128)
            ]
            return thunks

        # Startup DMA choreography: the serial DMA fill gates chunk 0, so
        # issue in consumption order. Steady state: each attention span's
        # fillers carry the NEXT chunk's projections and the PREVIOUS
        # chunk's cproj tiles, so the ACT-bound j-loop keeps the PE fed.
        nc.gpsimd.memset(ktf[64:128, :, :, :], 0)
        nc.sync.dma_start(w_q[:, 0:KO // 2, :], wq_r[:, 0:KO // 2, :])
        hi0 = issue_part(0, "hi")
        nc.sync.dma_start(w_q[:, KO // 2:, :], wq_r[:, KO // 2:, :])
        nc.sync.dma_start(ctab[:, :, 0:2 * TCH], ct_d[:, 0:2 * TCH])
        nc.sync.dma_start(stab[:, :, 0:2 * TCH], st_d[:, 0:2 * TCH])
        nc.sync.dma_start(w_k[:], wk_r)
        nc.sync.dma_start(w_vh[:], wvh_r)
        nc.sync.dma_start(w_vl[:], wvl_r)
        nc.sync.dma_start(tri[:], tri_d[:])
        nc.sync.dma_start(ones[:], ones_d[:])
        qc0, tq0 = proj_q_thunks(0, (hi0, None), cp_act=True)
        for t in tq0:
            t()
        lo0 = issue_part(0, "lo")
        for t in proj_kv_thunks(0, (hi0, lo0), cp_act=True):
            t()
        hi1 = issue_part(1, "hi")
        nc.sync.dma_start(ctab[:, :, 2 * TCH:], ct_d[:, 2 * TCH:])
        nc.sync.dma_start(stab[:, :, 2 * TCH:], st_d[:, 2 * TCH:])
        lo1 = issue_part(1, "lo")
        qc1, tq1 = proj_q_thunks(1, (hi1, None), cp_act=True)
        for t in tq1:
            t()
        ps_ = {1: (hi1, lo1), 2: issue_x(2)}
        qcs = [qc0, qc1]
        y8s = {}
        for c in range(N_CH):
            fillers = []
            if c + 1 < N_CH:
                fillers += proj_kv_thunks(c + 1, ps_[c + 1])
            if c + 2 < N_CH:
                qc_n, tq = proj_q_thunks(c + 2, ps_[c + 2])
                qcs.append(qc_n)
                fillers += tq
            if c - 1 in y8s:
                fillers += cproj_thunks((c - 1) * TCH, y8s.pop(c - 1),
                                        load_w=(c == 1))
            yc = ypool.tile([128, H_LOC, TCH], f32, tag="yc", name="yc")
            y8s[c] = attn_span(c * TCH, TCH, qcs[c], 0, yc, fillers=fillers)
            if c + 3 < N_CH:
                ps_[c + 3] = issue_x(c + 3)
        for t in cproj_thunks((N_CH - 1) * TCH, y8s.pop(N_CH - 1),
                              split_copies=True):
            t()
    nc.compile()
    _CACHE["nc"] = nc
    return nc


def host_inputs(x, Wq, Wk, Wv, Wo):
    """Per-core input dicts (host-side shard + transpose + fp8 split)."""
    import ml_dtypes

    F8 = ml_dtypes.float8_e4m3
    BF = ml_dtypes.bfloat16

    def f8_of(a):
        return np.asarray(a, np.float32).astype(F8)

    def f8_split(a):
        hi = f8_of(a)
        lo = (np.asarray(a, np.float32) - hi.astype(np.float32)).astype(F8)
        return hi, lo

    x2 = np.ascontiguousarray(x.reshape(T, D).T).astype(np.float32)  # (D, T)
    x8h, x8l = f8_split(x2)

    af = (1.0 / 1024.0) ** np.linspace(0.0, 1.0, DH // 4, dtype=np.float32)
    af = np.concatenate([af, np.zeros(DH // 4, np.float32)])         # (64,)
    theta = np.arange(T, dtype=np.float32)[:, None] * af[None, :]    # (T, 64)
    cos = np.cos(theta).T.astype(np.float32)                         # (64, T)
    sin = np.sin(theta).T.astype(np.float32)
    ctab = np.concatenate([cos, cos], axis=0).astype(BF)             # (128, T)
    stab = np.concatenate([sin, -sin], axis=0).astype(BF)

    ones = np.full((128, 128), 1.0 / YSC, BF)
    tri = np.triu(np.ones((128, 128), np.float32)).astype(BF)  # tri[j,i]=i>=j

    def wlay(a):
        # (KO*128, m) -> partition-major (128, KO*m) so one DMA loads it
        m = a.shape[1]
        return np.ascontiguousarray(
            a.reshape(KO, 128, m).transpose(1, 0, 2).reshape(128, KO * m))

    def olay(a):
        # (H_LOC*128, D) -> (128, H_LOC*D)
        return np.ascontiguousarray(
            a.reshape(H_LOC, 128, D).transpose(1, 0, 2).reshape(128, H_LOC * D))

    shared = {
        "x8h": x8h, "x8l": x8l, "ctab": ctab, "stab": stab,
        "ones": ones, "tri": tri,
    }
    in_maps = []
    for c in range(N_CORES):
        sl = slice(c * HD_LOC, (c + 1) * HD_LOC)
        wv8h, wv8l = f8_split(Wv[sl, :].T * WSC)
        wo8h, wo8l = f8_split((Wo[:, sl] / 3.0).T * WSC)
        in_maps.append({
            **shared,
            "wq8": wlay(f8_of(Wq[sl, :].T * WSC)),
            "wk8": wlay(f8_of(Wk[sl, :].T * WSC)),
            "wv8h": wlay(wv8h), "wv8l": wlay(wv8l),
            "wo8h": olay(wo8h), "wo8l": olay(wo8l),
        })
    return in_maps


def _get_runner():
    """Build the program + a persistent jitted SPMD executable (once)."""
    if "runner" in _CACHE:
        return _CACHE["runner"]

    import jax
    import concourse.mybir as mybir
    from concourse.bass2jax import (
        _bass_exec_p,
        install_neuronx_cc_hook,
        partition_id_tensor,
    )
    from jax.experimental.shard_map import shard_map
    from jax.sharding import Mesh, PartitionSpec

    nc = build_program()
    install_neuronx_cc_hook()
    assert nc.dbg_addr is None
    pid_name = nc.partition_id_tensor.name if nc.partition_id_tensor else None

    in_names, out_names, out_avals, zero_outs = [], [], [], []
    for alloc in nc.m.functions[0].allocations:
        if not isinstance(alloc, mybir.MemoryLocationSet):
            continue
        name = alloc.memorylocations[0].name
        if alloc.kind == "ExternalInput":
            if name != pid_name:
                in_names.append(name)
        elif alloc.kind == "ExternalOutput":
            out_names.append(name)
            shape = tuple(alloc.tensor_shape)
            dtype = mybir.dt.np(alloc.dtype)
            out_avals.append(jax.core.ShapedArray(shape, dtype))
            zero_outs.append(np.zeros(shape, dtype))
    n_params = len(in_names)
    all_names = list(in_names) + list(out_names)
    if pid_name is not None:
        all_names.append(pid_name)
    donate = tuple(range(n_params, n_params + len(out_names)))

    def _body(*args):
        operands = list(args)
        if pid_name is not None:
            operands.append(partition_id_tensor())
        outs = _bass_exec_p.bind(
            *operands,
            out_avals=tuple(out_avals),
            in_names=tuple(all_names),
            out_names=tuple(out_names),
            lowering_input_output_aliases=(),
            sim_require_finite=True,
            sim_require_nnan=True,
            nc=nc,
        )
        return tuple(outs)

    devices = jax.devices()[:N_CORES]
    mesh = Mesh(np.asarray(devices), ("core",))
    in_specs = (PartitionSpec("core"),) * (n_params + len(out_names))
    out_specs = (PartitionSpec("core"),) * len(out_names)
    fn = jax.jit(
        shard_map(_body, mesh=mesh, in_specs=in_specs, out_specs=out_specs,
                  check_rep=False),
        donate_argnums=donate,
        keep_unused=True,
    )
    runner = (fn, in_names, out_names, out_avals, zero_outs)
    _CACHE["runner"] = runner
    return runner


def run_spmd(in_maps):
    """Execute the SPMD program; returns per-core output dicts."""
    fn, in_names, out_names, out_avals, zero_outs = _get_runner()
    concat_in = [
        np.concatenate([np.asarray(in_maps[c][n]) for c in range(N_CORES)], axis=0)
        for n in in_names
    ]
    concat_zeros = [
        np.zeros((N_CORES * z.shape[0], *z.shape[1:]), z.dtype) for z in zero_outs
    ]
    out_arrs = fn(*concat_in, *concat_zeros)
    return [
        {n: np.asarray(out_arrs[i]).reshape(N_CORES, *out_avals[i].shape)[c]
         for i, n in enumerate(out_names)}
        for c in range(N_CORES)
    ]


def kernel(x, Wq, Wk, Wv, Wo):
    in_maps = host_inputs(np.asarray(x), np.asarray(Wq), np.asarray(Wk),
                          np.asarray(Wv), np.asarray(Wo))
    results = run_spmd(in_maps)
    out = results[0]["outp"].astype(np.float64)
    for c in range(1, N_CORES):
        out += results[c]["outp"].astype(np.float64)
    return out.astype(np.float32).reshape(1, T, D)
